# revision 3
# baseline (speedup 1.0000x reference)
"""CopyAttention (copy-generator head) Trainium2 kernel.

Full computation (see reference):
  p_copy  = sigmoid(q @ W_copy + b_copy)                       [B,T,1]
  prob    = softmax(set_pad(q @ W_gen + b_gen))                [B,T,V]
  ori     = prob * (1 - p_copy)
  attn    = softmax(mask(qW_in @ mem^T))                       [B,T,S]
  copy    = (attn * p_copy) @ src_map                          [B,T,E]
  out     = concat([ori, copy], -1)                            [B,T,V+E]

Sharding: data-parallel over the 2048 (B*T) token rows across 8 cores
(256 tokens each; core c -> batch c//2, half c%2). No collectives.

Per-core kernel:
  - generator matmul in fp16 (W_gen host-cast), PSUM f32 accumulation.
  - softmax over V without max-subtraction (logits ~ N(0,1); exp range is
    safe) -> exp written once to a f16 SBUF buffer with fused per-row
    accumulation (ACT accum_out), then a single scale pass multiplies by
    (1-p_copy)/sum and streams out.
  - attention path entirely in fp32 on the PE (scores ~ N(0, 1024) make
    low-precision matmuls unacceptable there), masked, max-subtracted
    softmax; unnormalized exp(attn) transposed on the PE and multiplied
    against fp16 src_map (exact 0/1 indicator); p_copy/sum folded in after.
"""
import sys

if "/opt/trn_rl_repo" not in sys.path:
    sys.path.insert(0, "/opt/trn_rl_repo")

from contextlib import ExitStack

import numpy as np

import concourse.bass as bass
import concourse.bacc as bacc
import concourse.tile as tile
from concourse import mybir
from concourse.bass_utils import run_bass_kernel_spmd
from concourse.masks import make_identity

AF = mybir.ActivationFunctionType
ALU = mybir.AluOpType
F32 = mybir.dt.float32
F16 = mybir.dt.float16

B, T, D, S, V, E = 4, 512, 1024, 512, 32000, 512
P = 128
KC = D // P              # 8 contraction chunks
TPC = 256                # tokens per core
NT = TPC // P            # 2 token tiles per core
NEG_INF = -1e18
EPS = 1e-20

# vocab groups: 31 x 1024 + 1 x 256
VG = 1024
VGROUPS = [(g * VG, VG) for g in range(V // VG)] + (
    [(V - V % VG, V % VG)] if V % VG else [])
NG = len(VGROUPS)


def build(has_bgen: bool):
    nc = bacc.Bacc("TRN2", target_bir_lowering=False, debug=False, num_devices=8)

    qT32 = nc.dram_tensor("qT32", [D, TPC], F32, kind="ExternalInput")
    qT16 = nc.dram_tensor("qT16", [D, TPC], F16, kind="ExternalInput")
    w_in = nc.dram_tensor("w_in", [D, D], F32, kind="ExternalInput")
    w_copy = nc.dram_tensor("w_copy", [D, 1], F32, kind="ExternalInput")
    b_copy = nc.dram_tensor("b_copy", [1], F32, kind="ExternalInput")
    w16 = nc.dram_tensor("w16", [D, V], F16, kind="ExternalInput")
    memT = nc.dram_tensor("memT", [D, S], F32, kind="ExternalInput")
    smap = nc.dram_tensor("smap", [S, E], F16, kind="ExternalInput")
    maskadd = nc.dram_tensor("maskadd", [S], F32, kind="ExternalInput")
    bgen = (nc.dram_tensor("bgen", [V], F16, kind="ExternalInput")
            if has_bgen else None)
    out = nc.dram_tensor("out", [TPC, V + E], F32, kind="ExternalOutput")

    qT32_r = qT32.ap().rearrange("(c p) t -> p c t", p=P)
    qT16_r = qT16.ap().rearrange("(c p) t -> p c t", p=P)
    w16_r = w16.ap().rearrange("(c p) v -> p c v", p=P)
    w_in_r = w_in.ap().rearrange("(c p) d -> p c d", p=P)
    memT_r = memT.ap().rearrange("(c p) s -> p c s", p=P)
    smap_r = smap.ap().rearrange("(c p) e -> p c e", p=P)
    wc_r = w_copy.ap().rearrange("(c p) o -> p c o", p=P)
    out_ap = out.ap()

    with tile.TileContext(nc) as tc, ExitStack() as ctx:
        persist = ctx.enter_context(tc.tile_pool(name="persist", bufs=1))
        small = ctx.enter_context(tc.tile_pool(name="small", bufs=1))

        # resident tensors
        qT16_t = persist.tile([P, KC, TPC], F16)          # 4 KB/p
        qT32_t = persist.tile([P, KC, TPC], F32)          # 8 KB/p
        expst = persist.tile([P, NT, V], F16)             # 125 KB/p
        nc.sync.dma_start(qT16_t[:], qT16_r)
        nc.sync.dma_start(qT32_t[:], qT32_r)

        wc_t = small.tile([P, KC, 1], F32)
        bc_t = small.tile([P, 1], F32)
        mask_t = small.tile([P, S], F32)
        smap_t = small.tile([P, S // P, E], F16)
        ident = small.tile([P, P], F16)
        nc.sync.dma_start(wc_t[:], wc_r)
        nc.sync.dma_start(bc_t[:], bass.AP(tensor=b_copy, offset=0, ap=[[0, P], [1, 1]]))
        nc.sync.dma_start(mask_t[:], bass.AP(tensor=maskadd, offset=0, ap=[[0, P], [1, S]]))
        nc.sync.dma_start(smap_t[:], smap_r)
        make_identity(nc, ident[:])

        pc_t = small.tile([P, NT], F32)       # p_copy per token tile
        ompc_t = small.tile([P, NT], F32)     # 1 - p_copy
        gsums = small.tile([P, NT, NG], F32)  # per-group exp row sums
        cgen = small.tile([P, NT], F32)       # (1-p_copy)/sum_gen
        c2 = small.tile([P, NT], F32)         # p_copy/sum_attn
        asum = small.tile([P, NT], F32)

        # ---------------- p_copy (tiny, runs first) ----------------
        with tc.tile_pool(name="pc_ps", bufs=1, space="PSUM") as pcp:
            for t in range(NT):
                ps = pcp.tile([P, 1], F32, tag=f"pc{t}")
                for k in range(KC):
                    nc.tensor.matmul(ps[:], qT32_t[:, k, t * P:(t + 1) * P],
                                     wc_t[:, k, :], start=(k == 0), stop=(k == KC - 1))
                nc.scalar.activation(pc_t[:, t:t + 1], ps[:], AF.Sigmoid,
                                     bias=bc_t[:])
                nc.vector.tensor_scalar(ompc_t[:, t:t + 1], pc_t[:, t:t + 1],
                                        -1.0, 1.0, ALU.mult, ALU.add)

        # ---------------- generator matmul + exp ----------------
        with tc.tile_pool(name="wg", bufs=2) as wpool, \
             tc.tile_pool(name="gps", bufs=2, space="PSUM") as gpsum, \
             (tc.tile_pool(name="bg", bufs=2) if has_bgen else ExitStack()) as bgpool:
            for g, (v0, vg) in enumerate(VGROUPS):
                wt = wpool.tile([P, KC, VG], F16, tag="wt")
                nc.sync.dma_start(wt[:, :, :vg], w16_r[:, :, v0:v0 + vg])
                if has_bgen:
                    bg_t = bgpool.tile([1, VG], F16, tag="bg")
                    nc.sync.dma_start(bg_t[:1, :vg], bgen.ap()[v0:v0 + vg].unsqueeze(0))
                for t in range(NT):
                    ps = gpsum.tile([P, VG], F32, tag=f"g{t}")
                    for h in range(0, vg, 512):
                        hw = min(512, vg - h)
                        for k in range(KC):
                            nc.tensor.matmul(
                                ps[:, h:h + hw], qT16_t[:, k, t * P:(t + 1) * P],
                                wt[:, k, h:h + hw],
                                start=(k == 0), stop=(k == KC - 1))
                    if has_bgen:
                        nc.vector.tensor_tensor(
                            out=ps[:, :vg], in0=ps[:, :vg],
                            in1=bg_t[0:1, :vg].to_broadcast((P, vg)), op=ALU.add)
                    if g == 0:
                        # PAD column: reference sets logit[PAD] = -1e-20
                        nc.vector.memset(ps[:, 0:1], -EPS)
                    nc.scalar.activation(
                        expst[:, t, v0:v0 + vg], ps[:, :vg], AF.Exp,
                        accum_out=gsums[:, t, g:g + 1])

        # sums -> cgen = (1 - p_copy) / sum
        for t in range(NT):
            nc.vector.tensor_reduce(cgen[:, t:t + 1], gsums[:, t, :],
                                    op=ALU.add, axis=mybir.AxisListType.X)
            nc.vector.reciprocal(cgen[:, t:t + 1], cgen[:, t:t + 1])
            nc.vector.tensor_mul(cgen[:, t:t + 1], cgen[:, t:t + 1],
                                 ompc_t[:, t:t + 1])

        # ---------------- attention path (fp32) ----------------
        attn16 = small.tile([P, NT, S], F16)
        attnT = small.tile([P, S // P, TPC], F16)
        qint = persist.tile([P, KC, TPC], F32)  # (q @ W_in)^T, 8 KB/p

        with tc.tile_pool(name="winp", bufs=3) as winp, \
             tc.tile_pool(name="qinps", bufs=1, space="PSUM") as qinps:
            qps = qinps.tile([P, KC, 512], F32)  # one full bank per d-group  # 4 banks
            for k in range(KC):
                wk = winp.tile([P, D], F32, tag="wk")
                nc.sync.dma_start(wk[:], w_in_r[:, k, :])
                for d in range(KC):
                    nc.tensor.matmul(qps[:, d, :TPC], wk[:, d * P:(d + 1) * P],
                                     qT32_t[:, k, :],
                                     start=(k == 0), stop=(k == KC - 1))
            nc.scalar.copy(qint[:], qps[:, :, :TPC])

        with tc.tile_pool(name="memp", bufs=3) as memp, \
             tc.tile_pool(name="aps", bufs=1, space="PSUM") as apsum, \
             tc.tile_pool(name="scb", bufs=1) as scb:
            sc_ps = [apsum.tile([P, S], F32, tag=f"sc{t}", name=f"sc_ps{t}")
                     for t in range(NT)]
            for k in range(KC):
                mk = memp.tile([P, S], F32, tag="mk")
                nc.sync.dma_start(mk[:], memT_r[:, k, :])
                for t in range(NT):
                    nc.tensor.matmul(sc_ps[t][:], qint[:, k, t * P:(t + 1) * P],
                                     mk[:], start=(k == 0), stop=(k == KC - 1))
            for t in range(NT):
                scores = scb.tile([P, S], F32, tag=f"scores{t}")
                negmax = small.tile([P, 1], F32, tag=f"negmax{t}")
                nc.vector.tensor_add(scores[:], sc_ps[t][:], mask_t[:])
                nc.vector.tensor_reduce(negmax[:], scores[:], op=ALU.max,
                                        axis=mybir.AxisListType.X)
                nc.vector.tensor_scalar_mul(negmax[:], negmax[:], -1.0)
                nc.scalar.activation(attn16[:, t, :], scores[:], AF.Exp,
                                     bias=negmax[:],
                                     accum_out=asum[:, t:t + 1])

            # transpose attn (PE) and scatter-matmul against src_map
            for t in range(NT):
                for sc in range(S // P):
                    tp = apsum.tile([P, P], F16, tag="tp")
                    nc.tensor.transpose(tp[:], attn16[:, t, sc * P:(sc + 1) * P],
                                        ident[:])
                    nc.scalar.copy(attnT[:, sc, t * P:(t + 1) * P], tp[:])
            for t in range(NT):
                cps = apsum.tile([P, E], F32, tag=f"cp{t}")
                for sc in range(S // P):
                    nc.tensor.matmul(cps[:], attnT[:, sc, t * P:(t + 1) * P],
                                     smap_t[:, sc, :],
                                     start=(sc == 0), stop=(sc == S // P - 1))
                nc.vector.reciprocal(c2[:, t:t + 1], asum[:, t:t + 1])
                nc.vector.tensor_mul(c2[:, t:t + 1], c2[:, t:t + 1], pc_t[:, t:t + 1])
                cob = scb.tile([P, E], F32, tag=f"co{t}")
                nc.vector.tensor_scalar_mul(cob[:], cps[:], c2[:, t:t + 1])
                nc.sync.dma_start(out_ap[t * P:(t + 1) * P, V:V + E], cob[:])

        # ---------------- generator output scale pass ----------------
        with tc.tile_pool(name="stage", bufs=4) as stp:
            for t in range(NT):
                for g, (v0, vg) in enumerate(VGROUPS):
                    st = stp.tile([P, VG], F32, tag="st")
                    nc.vector.tensor_scalar_mul(st[:, :vg], expst[:, t, v0:v0 + vg],
                                                cgen[:, t:t + 1])
                    nc.sync.dma_start(out_ap[t * P:(t + 1) * P, v0:v0 + vg],
                                      st[:, :vg])

    nc.compile()
    return nc


_CACHE = {}


def _get_nc(has_bgen: bool):
    if has_bgen not in _CACHE:
        _CACHE[has_bgen] = build(has_bgen)
    return _CACHE[has_bgen]


def _prep_in_maps(query, memory_bank, src_pad_mask, src_map, W_in, W_copy,
                  b_copy, W_gen, b_gen):
    w16 = np.ascontiguousarray(W_gen, dtype=np.float32).astype(np.float16)
    has_bgen = bool(np.any(b_gen))
    w_in = np.ascontiguousarray(W_in, dtype=np.float32)
    w_copy = np.ascontiguousarray(W_copy, dtype=np.float32).reshape(D, 1)
    b_copy = np.ascontiguousarray(b_copy, dtype=np.float32).reshape(1)
    bgen16 = b_gen.astype(np.float16)

    in_maps = []
    for c in range(8):
        b, h = c // 2, c % 2
        q = np.ascontiguousarray(query[b, h * TPC:(h + 1) * TPC, :], dtype=np.float32)
        qT = np.ascontiguousarray(q.T)
        m = {
            "qT32": qT,
            "qT16": qT.astype(np.float16),
            "w_in": w_in,
            "w_copy": w_copy,
            "b_copy": b_copy,
            "w16": w16,
            "memT": np.ascontiguousarray(memory_bank[b].T, dtype=np.float32),
            "smap": np.ascontiguousarray(src_map[b], dtype=np.float32).astype(np.float16),
            "maskadd": np.where(src_pad_mask[b], np.float32(NEG_INF),
                                np.float32(0.0)).astype(np.float32),
        }
        if has_bgen:
            m["bgen"] = bgen16
        in_maps.append(m)
    return in_maps, has_bgen


def run(trace=False, trace_cores=None, **inputs):
    in_maps, has_bgen = _prep_in_maps(**{
        k: np.asarray(v) for k, v in inputs.items()})
    nc = _get_nc(has_bgen)
    kw = {}
    if trace:
        kw = dict(trace=True, trace_cores=trace_cores or [0])
    res = run_bass_kernel_spmd(nc, in_maps, core_ids=list(range(8)), **kw)
    outs = np.concatenate([res.results[c]["out"] for c in range(8)], axis=0)
    return outs.reshape(B, T, V + E).astype(np.float32), res


def kernel(**inputs):
    out, _ = run(**inputs)
    return out
